# revision 7
# baseline (speedup 1.0000x reference)
"""Distributed Trainium2 kernel for the 2-layer GraphConv network.

Strategy (graph/data parallel, dst-partitioned):
- Host: compute degree norms from src/dst (integer index data), fold ALL
  normalization into per-edge weights w = norm_out[src]*norm_in[dst], which
  live in host-built per-chunk selector matrices S.  Edges are assigned to
  the core owning their dst node, grouped into 64-dst "sub-blocks", split by
  src table half (int16 DMA-gather index limit), and padded to 128-edge
  chunks with chunk budgets uniform across cores (SPMD: one instruction
  stream for all 8 cores).
- Device: AllGather the bf16 node-feature table; per call, dma_gather message
  rows, multiply msg_chunk^T @ S on the TensorEngine accumulating agg^T tiles
  in PSUM (this performs the segment-sum aggregation), then apply the dense
  layers (W1+bias+relu, W2) per 256-column chunk, transpose, AllGather the
  layer-2 table, repeat message passing, add b2 + residual x, write out.
"""

import os
import sys

import numpy as np

sys.path.insert(0, os.path.dirname(os.path.abspath(__file__)))

N = 50000
E = 800000
F = 128
H = 256
NCORES = 8
RPC = N // NCORES          # 6250 rows per core
CHUNK = 128
RPAD = 6272                # 49 * 128, per-core padded row count
NROWT = RPAD * NCORES      # 50176 rows in the gathered table
HALFR = NROWT // 2         # 25088, int16-addressable half
SEG = 64                   # dst slots per sub-block
SB_PER_CALL = 4
CALL_COLS = SEG * SB_PER_CALL   # 256
NSB = RPAD // SEG          # 98 sub-blocks per core
NCALLS = (NSB + SB_PER_CALL - 1) // SB_PER_CALL  # 25

DT_NAME = os.environ.get("GNN_DT", "bf16")


class Call:
    __slots__ = ("nA", "nB", "idx_off", "ch_off", "sb_slots", "evict_cols")


class Plan:
    __slots__ = ("calls", "totch", "icols", "slot_base", "budA", "budB")


def _norms(src, dst):
    deg_out = np.bincount(src, minlength=N).astype(np.float32)
    deg_in = np.bincount(dst, minlength=N).astype(np.float32)
    norm_out = (1.0 / np.sqrt(np.clip(deg_out, 1.0, None))).astype(np.float32)
    norm_in = (1.0 / np.sqrt(np.clip(deg_in, 1.0, None))).astype(np.float32)
    return norm_out, norm_in


def make_plan(src, dst):
    """Compute uniform (cross-core) chunk budgets and the call schedule."""
    src = np.asarray(src).astype(np.int64)
    dst = np.asarray(dst).astype(np.int64)
    owner = dst // RPC
    budA = np.zeros(NSB, np.int64)
    budB = np.zeros(NSB, np.int64)
    per_core_groups = []
    for c in range(NCORES):
        m = owner == c
        ed = dst[m] - c * RPC
        es = src[m]
        ps = (es // RPC) * RPAD + (es % RPC)
        half = (ps >= HALFR).astype(np.int64)
        sbid = ed // SEG
        cntA = np.bincount(sbid[half == 0], minlength=NSB)
        cntB = np.bincount(sbid[half == 1], minlength=NSB)
        budA = np.maximum(budA, -(-cntA // CHUNK))
        budB = np.maximum(budB, -(-cntB // CHUNK))
        per_core_groups.append((ed, es, ps, half, sbid, m))
    # make sure every sub-block has at least one chunk so its PSUM region is
    # written (start=True zeroes it) even when no core has edges there
    empty = (budA + budB) == 0
    budA[empty] = 1

    plan = Plan()
    plan.budA, plan.budB = budA, budB
    plan.calls = []
    plan.slot_base = {}
    slot = 0
    choff = 0
    for k in range(NCALLS):
        sbs = list(range(k * SB_PER_CALL, min((k + 1) * SB_PER_CALL, NSB)))
        call = Call()
        call.idx_off = slot
        call.ch_off = choff
        call.evict_cols = len(sbs) * SEG
        a_slots = {}
        local = 0
        for sb in sbs:
            plan.slot_base[(sb, 0)] = slot + local * CHUNK
            a_slots[sb] = list(range(local, local + budA[sb]))
            local += budA[sb]
        call.nA = local
        for sb in sbs:
            plan.slot_base[(sb, 1)] = slot + local * CHUNK
            a_slots[sb] = a_slots[sb] + list(range(local, local + budB[sb]))
            local += budB[sb]
        call.nB = local - call.nA
        call.sb_slots = [a_slots[sb] for sb in sbs]
        slot += local * CHUNK
        choff += local
        plan.calls.append(call)
    plan.totch = choff
    plan.icols = slot // 16
    return plan, per_core_groups


def make_core_arrays(plan, groups, w_all, src, dst, np_dt):
    """Per-core idx stream (int16, 16-wrapped+replicated) and selector S."""
    out = []
    totslots = plan.totch * CHUNK
    for c in range(NCORES):
        ed, es, ps, half, sbid, m = groups[c]
        w = w_all[m]
        psl = np.where(half == 1, ps - HALFR, ps)
        key = sbid * 2 + half
        order = np.argsort(key, kind="stable")
        ed_s, psl_s, w_s, key_s = ed[order], psl[order], w[order], key[order]
        bounds = np.searchsorted(key_s, np.arange(2 * NSB + 1))
        idx_flat = np.zeros(totslots, np.int16)
        col_flat = np.zeros(totslots, np.int64)
        w_flat = np.zeros(totslots, np.float32)
        for sb in range(NSB):
            for h in (0, 1):
                g0, g1 = bounds[2 * sb + h], bounds[2 * sb + h + 1]
                n = g1 - g0
                if n == 0:
                    continue
                base = plan.slot_base[(sb, h)]
                idx_flat[base : base + n] = psl_s[g0:g1].astype(np.int16)
                col_flat[base : base + n] = ed_s[g0:g1] - sb * SEG
                w_flat[base : base + n] = w_s[g0:g1]
        wrapped = idx_flat.reshape(-1, 16).T.copy()      # [16, icols]
        idx_arr = np.tile(wrapped, (8, 1))               # replicate per Q7 core
        S = np.zeros((totslots, SEG), np.float32)
        S[np.arange(totslots), col_flat] = w_flat
        S = S.reshape(plan.totch, CHUNK, SEG).astype(np_dt)
        out.append((idx_arr, S))
    return out


def build_graph(plan, dt_name):
    import concourse.bacc as bacc
    import concourse.mybir as mybir
    import concourse.tile as tile

    f32 = mybir.dt.float32
    DT = mybir.dt.bfloat16 if dt_name == "bf16" else mybir.dt.float32
    chcap = max(c.nA + c.nB for c in plan.calls)

    nc = bacc.Bacc("TRN2", target_bir_lowering=False, debug=False,
                   num_devices=NCORES)
    xs_p = nc.dram_tensor("xs", [RPAD, F], f32, kind="ExternalInput")
    idx_p = nc.dram_tensor("idx", [128, plan.icols], mybir.dt.int16,
                           kind="ExternalInput")
    sel_p = nc.dram_tensor("sel", [plan.totch, CHUNK, SEG], DT,
                           kind="ExternalInput")
    w1_p = nc.dram_tensor("w1", [F, H], f32, kind="ExternalInput")
    w2_p = nc.dram_tensor("w2", [H, F], f32, kind="ExternalInput")
    b1_p = nc.dram_tensor("b1", [2, 128], f32, kind="ExternalInput")
    b2_p = nc.dram_tensor("b2", [1, 128], f32, kind="ExternalInput")
    id_p = nc.dram_tensor("ident", [128, 128], f32, kind="ExternalInput")
    out_p = nc.dram_tensor("out", [RPAD, F], f32, kind="ExternalOutput")

    add = mybir.AluOpType.add
    mx = mybir.AluOpType.max
    rg = [list(range(NCORES))]

    with tile.TileContext(nc) as tc:
        with (
            tc.tile_pool(name="const", bufs=1) as constp,
            tc.tile_pool(name="res", bufs=1) as resp,
            tc.tile_pool(name="msg", bufs=2) as msgp,
            tc.tile_pool(name="selp", bufs=2) as selpool,
            tc.tile_pool(name="stage", bufs=2) as stagep,
            tc.tile_pool(name="ps_mp", bufs=2, space="PSUM") as psmp,
            tc.tile_pool(name="ps_w", bufs=3, space="PSUM") as pswp,
            tc.tile_pool(name="ps_t", bufs=1, space="PSUM") as pstp,
            tc.tile_pool(name="dram", bufs=1, space="DRAM") as dram,
        ):
            # ---- constants / resident tensors ----
            xs_t = resp.tile([128, 49, F], f32, tag="xs")
            nc.sync.dma_start(xs_t[:], xs_p.ap().rearrange("(c p) f -> p c f", p=128))
            idx_t = resp.tile([128, plan.icols], mybir.dt.int16, tag="idx")
            nc.sync.dma_start(idx_t[:], idx_p[:, :])
            w1f = constp.tile([F, H], f32, tag="w1f")
            nc.sync.dma_start(w1f[:], w1_p[:, :])
            w2f = constp.tile([128, 2, 128], f32, tag="w2f")
            nc.sync.dma_start(w2f[:], w2_p.ap().rearrange("(s k) m -> k s m", k=128))
            b1c = constp.tile([128, 2], f32, tag="b1")
            nc.sync.dma_start(b1c[:], b1_p.ap().rearrange("h p -> p h"))
            b2c = constp.tile([128, 1], f32, tag="b2")
            nc.sync.dma_start(b2c[:], b2_p.ap().rearrange("h p -> p h"))
            idf = constp.tile([128, 128], f32, tag="idf")
            nc.sync.dma_start(idf[:], id_p[:, :])
            if DT != f32:
                w1d = constp.tile([F, H], DT, tag="w1d")
                nc.vector.tensor_copy(w1d[:], w1f[:])
                w2d = constp.tile([128, 2, 128], DT, tag="w2d")
                nc.vector.tensor_copy(w2d[:], w2f[:])
                idd = constp.tile([128, 128], DT, tag="idd")
                nc.vector.tensor_copy(idd[:], idf[:])
            else:
                w1d, w2d, idd = w1f, w2f, idf

            bounce1 = dram.tile([RPAD, F], DT, tag="bounce1")
            table1 = dram.tile([NROWT, F], DT, tag="table1", addr_space="Shared")
            bounce2 = dram.tile([RPAD, F], DT, tag="bounce2")
            table2 = dram.tile([NROWT, F], DT, tag="table2", addr_space="Shared")

            # bounce1 = cast(xs)
            xbf = resp.tile([128, 49, F], DT, tag="xbf")
            nc.vector.tensor_copy(xbf[:], xs_t[:])
            nc.sync.dma_start(
                bounce1[:, :].rearrange("(c p) f -> p c f", p=128), xbf[:]
            )
            nc.gpsimd.collective_compute(
                "AllGather", mybir.AluOpType.bypass, replica_groups=rg,
                ins=[bounce1.opt()], outs=[table1.opt()],
            )

            limit = int(os.environ.get("GNN_LIMIT", "9999"))
            no_gather = os.environ.get("GNN_NO_GATHER", "0") == "1"
            no_mm = os.environ.get("GNN_NO_MM", "0") == "1"

            def msgpass(table, layer):
                tabA = table[0:HALFR, :]
                tabB = table[HALFR:NROWT, :]
                for k, call in enumerate(plan.calls):
                    if k >= limit:
                        break
                    nch = call.nA + call.nB
                    ecols = call.evict_cols
                    nct = ecols // 128  # output 128-col chunks (2, last call 1)
                    msg = msgp.tile([128, chcap, F], DT, tag="msg")
                    st = selpool.tile([128, chcap, SEG], DT, tag="sel")
                    nc.sync.dma_start(
                        st[:, 0:nch, :],
                        sel_p[call.ch_off : call.ch_off + nch, :, :].rearrange(
                            "t p s -> p t s"
                        ),
                    )
                    if call.nA and not no_gather:
                        nc.gpsimd.dma_gather(
                            out_ap=msg[:, 0 : call.nA, :],
                            in_ap=tabA,
                            idxs_ap=idx_t[
                                :, call.idx_off // 16 : (call.idx_off + call.nA * 128) // 16
                            ],
                            num_idxs=call.nA * 128,
                            num_idxs_reg=call.nA * 128,
                            elem_size=F,
                            single_packet=call.nA * 128 <= 1024,
                        )
                    if call.nB and not no_gather:
                        boff = call.idx_off + call.nA * 128
                        nc.gpsimd.dma_gather(
                            out_ap=msg[:, call.nA : nch, :],
                            in_ap=tabB,
                            idxs_ap=idx_t[:, boff // 16 : (boff + call.nB * 128) // 16],
                            num_idxs=call.nB * 128,
                            num_idxs_reg=call.nB * 128,
                            elem_size=F,
                            single_packet=call.nB * 128 <= 1024,
                        )
                    ps = psmp.tile([128, CALL_COLS], f32, tag="mp")
                    if no_gather:
                        nc.vector.memset(msg[:, 0:nch, :], 0.25)
                    if no_mm:
                        nc.vector.memset(ps[:, :], 0.0)
                    for sb_i, slots in enumerate([] if no_mm else call.sb_slots):
                        for si, t in enumerate(slots):
                            nc.tensor.matmul(
                                ps[:, sb_i * SEG : (sb_i + 1) * SEG],
                                msg[:, t, :],
                                st[:, t, :],
                                start=(si == 0),
                                stop=(si == len(slots) - 1),
                            )
                    if layer == 1:
                        agg = stagep.tile([128, CALL_COLS], DT, tag="agg")
                        nc.any.tensor_copy(agg[:, :ecols], ps[:, :ecols])
                        h0 = stagep.tile([128, CALL_COLS], DT, tag="h0")
                        h1 = stagep.tile([128, CALL_COLS], DT, tag="h1")
                        for hf, ht in ((0, h0), (1, h1)):
                            wp = pswp.tile([128, CALL_COLS], f32, tag="wp")
                            nc.tensor.matmul(
                                wp[:, :ecols],
                                w1d[:, hf * 128 : (hf + 1) * 128],
                                agg[:, :ecols],
                                start=True,
                                stop=True,
                            )
                            nc.any.tensor_scalar(
                                ht[:, :ecols], wp[:, :ecols],
                                b1c[:, hf : hf + 1], 0.0, op0=add, op1=mx,
                            )
                        wp2 = pswp.tile([128, CALL_COLS], f32, tag="wp")
                        nc.tensor.matmul(
                            wp2[:, :ecols], w2d[:, 0, :], h0[:, :ecols],
                            start=True, stop=False,
                        )
                        nc.tensor.matmul(
                            wp2[:, :ecols], w2d[:, 1, :], h1[:, :ecols],
                            start=False, stop=True,
                        )
                        g = stagep.tile([128, CALL_COLS], DT, tag="g")
                        nc.any.tensor_copy(g[:, :ecols], wp2[:, :ecols])
                        gr = stagep.tile([128, SB_PER_CALL // 2, F], DT, tag="gr")
                        for ci in range(nct):
                            tp = pstp.tile([128, 128], DT, tag="tpd")
                            nc.tensor.transpose(
                                tp[:], g[:, ci * 128 : (ci + 1) * 128], idd[:]
                            )
                            nc.any.tensor_copy(gr[:, ci, :], tp[:])
                        nc.sync.dma_start(
                            bounce2[
                                k * CALL_COLS : k * CALL_COLS + ecols, :
                            ].rearrange("(c p) f -> p c f", p=128),
                            gr[:, 0:nct, :],
                        )
                    else:
                        a2 = stagep.tile([128, CALL_COLS], f32, tag="a2")
                        nc.any.tensor_scalar_add(
                            a2[:, :ecols], ps[:, :ecols], b2c[:, 0:1]
                        )
                        orow = stagep.tile([128, SB_PER_CALL // 2, F], f32, tag="or")
                        for ci in range(nct):
                            tp = pstp.tile([128, 128], f32, tag="tp")
                            nc.tensor.transpose(
                                tp[:], a2[:, ci * 128 : (ci + 1) * 128], idf[:]
                            )
                            cg = k * (CALL_COLS // 128) + ci
                            nc.vector.tensor_add(
                                orow[:, ci, :], tp[:], xs_t[:, cg, :]
                            )
                        nc.sync.dma_start(
                            out_p.ap()[
                                k * CALL_COLS : k * CALL_COLS + ecols, :
                            ].rearrange("(c p) f -> p c f", p=128),
                            orow[:, 0:nct, :],
                        )

            msgpass(table1, 1)
            nc.gpsimd.collective_compute(
                "AllGather", mybir.AluOpType.bypass, replica_groups=rg,
                ins=[bounce2.opt()], outs=[table2.opt()],
            )
            msgpass(table2, 2)

    nc.compile()
    return nc


def prepare(x, W1, b1, W2, b2, src, dst, dt_name=DT_NAME):
    import concourse.mybir as mybir

    np_dt = mybir.dt.np(
        mybir.dt.bfloat16 if dt_name == "bf16" else mybir.dt.float32
    )
    src = np.asarray(src).astype(np.int64)
    dst = np.asarray(dst).astype(np.int64)
    x = np.asarray(x, dtype=np.float32)
    norm_out, norm_in = _norms(src, dst)
    w_all = (norm_out[src] * norm_in[dst]).astype(np.float32)
    plan, groups = make_plan(src, dst)
    core_arrays = make_core_arrays(plan, groups, w_all, src, dst, np_dt)

    W1 = np.asarray(W1, dtype=np.float32)
    W2 = np.asarray(W2, dtype=np.float32)
    b1 = np.asarray(b1, dtype=np.float32).reshape(2, 128)
    b2 = np.asarray(b2, dtype=np.float32).reshape(1, 128)
    ident = np.eye(128, dtype=np.float32)

    in_maps = []
    for c in range(NCORES):
        xs = np.zeros((RPAD, F), np.float32)
        xs[:RPC] = x[c * RPC : (c + 1) * RPC]
        idx_arr, S = core_arrays[c]
        in_maps.append(
            {
                "xs": xs,
                "idx": idx_arr,
                "sel": S,
                "w1": W1,
                "w2": W2,
                "b1": b1,
                "b2": b2,
                "ident": ident,
            }
        )
    return plan, in_maps


_CACHE = {}


def run(x, W1, b1, W2, b2, src, dst, trace=False, dt_name=DT_NAME):
    from concourse import bass_utils

    key = (int(np.asarray(src)[0]), int(np.asarray(dst)[-1]), dt_name)
    plan, in_maps = prepare(x, W1, b1, W2, b2, src, dst, dt_name)
    if key not in _CACHE:
        _CACHE[key] = build_graph(plan, dt_name)
    nc = _CACHE[key]
    res = bass_utils.run_bass_kernel_spmd(
        nc, in_maps, core_ids=list(range(NCORES)), trace=trace
    )
    out = np.concatenate([res.results[c]["out"][:RPC] for c in range(NCORES)])
    return out.astype(np.float32), res.exec_time_ns


def kernel(x, W1, b1, W2, b2, src, dst):
    out, _ = run(x, W1, b1, W2, b2, src, dst, trace=False)
    return out


# revision 8
# speedup vs baseline: 1.4391x; 1.4391x over previous
"""Distributed Trainium2 kernel for the 2-layer GraphConv network.

Strategy (graph/data parallel, dst-partitioned):
- Host: compute degree norms from src/dst (integer index data), fold ALL
  normalization into per-edge weights w = norm_out[src]*norm_in[dst], which
  live in host-built per-chunk selector matrices S.  Edges are assigned to
  the core owning their dst node, grouped into 64-dst "sub-blocks", split by
  src table half (int16 DMA-gather index limit), and padded to 128-edge
  chunks with chunk budgets uniform across cores (SPMD: one instruction
  stream for all 8 cores).
- Device: AllGather the bf16 node-feature table; per call, dma_gather message
  rows, multiply msg_chunk^T @ S on the TensorEngine accumulating agg^T tiles
  in PSUM (this performs the segment-sum aggregation), then apply the dense
  layers (W1+bias+relu, W2) per 256-column chunk, transpose, AllGather the
  layer-2 table, repeat message passing, add b2 + residual x, write out.
"""

import os
import sys

import numpy as np

sys.path.insert(0, os.path.dirname(os.path.abspath(__file__)))

N = 50000
E = 800000
F = 128
H = 256
NCORES = 8
RPC = N // NCORES          # 6250 rows per core
CHUNK = 128
RPAD = 6272                # 49 * 128, per-core padded row count
NROWT = RPAD * NCORES      # 50176 rows in the gathered table
HALFR = NROWT // 2         # 25088, int16-addressable half
SEG = 64                   # dst slots per sub-block
SB_PER_CALL = 4
CALL_COLS = SEG * SB_PER_CALL   # 256
NSB = RPAD // SEG          # 98 sub-blocks per core
NCALLS = (NSB + SB_PER_CALL - 1) // SB_PER_CALL  # 25

DT_NAME = os.environ.get("GNN_DT", "bf16")


class Call:
    __slots__ = ("nA", "nB", "idx_off", "ch_off", "sb_slots", "evict_cols")


class Plan:
    __slots__ = ("calls", "totch", "icols", "slot_base", "budA", "budB")


def _norms(src, dst):
    deg_out = np.bincount(src, minlength=N).astype(np.float32)
    deg_in = np.bincount(dst, minlength=N).astype(np.float32)
    norm_out = (1.0 / np.sqrt(np.clip(deg_out, 1.0, None))).astype(np.float32)
    norm_in = (1.0 / np.sqrt(np.clip(deg_in, 1.0, None))).astype(np.float32)
    return norm_out, norm_in


def make_plan(src, dst):
    """Compute uniform (cross-core) chunk budgets and the call schedule."""
    src = np.asarray(src).astype(np.int64)
    dst = np.asarray(dst).astype(np.int64)
    owner = dst // RPC
    budA = np.zeros(NSB, np.int64)
    budB = np.zeros(NSB, np.int64)
    per_core_groups = []
    for c in range(NCORES):
        m = owner == c
        ed = dst[m] - c * RPC
        es = src[m]
        ps = (es // RPC) * RPAD + (es % RPC)
        half = (ps >= HALFR).astype(np.int64)
        sbid = ed // SEG
        cntA = np.bincount(sbid[half == 0], minlength=NSB)
        cntB = np.bincount(sbid[half == 1], minlength=NSB)
        budA = np.maximum(budA, -(-cntA // CHUNK))
        budB = np.maximum(budB, -(-cntB // CHUNK))
        per_core_groups.append((ed, es, ps, half, sbid, m))
    # make sure every sub-block has at least one chunk so its PSUM region is
    # written (start=True zeroes it) even when no core has edges there
    empty = (budA + budB) == 0
    budA[empty] = 1

    plan = Plan()
    plan.budA, plan.budB = budA, budB
    plan.calls = []
    plan.slot_base = {}
    slot = 0
    choff = 0
    for k in range(NCALLS):
        sbs = list(range(k * SB_PER_CALL, min((k + 1) * SB_PER_CALL, NSB)))
        call = Call()
        call.idx_off = slot
        call.ch_off = choff
        call.evict_cols = len(sbs) * SEG
        a_slots = {}
        local = 0
        for sb in sbs:
            plan.slot_base[(sb, 0)] = slot + local * CHUNK
            a_slots[sb] = list(range(local, local + budA[sb]))
            local += budA[sb]
        call.nA = local
        for sb in sbs:
            plan.slot_base[(sb, 1)] = slot + local * CHUNK
            a_slots[sb] = a_slots[sb] + list(range(local, local + budB[sb]))
            local += budB[sb]
        call.nB = local - call.nA
        call.sb_slots = [a_slots[sb] for sb in sbs]
        slot += local * CHUNK
        choff += local
        plan.calls.append(call)
    plan.totch = choff
    plan.icols = slot // 16
    return plan, per_core_groups


def make_core_arrays(plan, groups, w_all, src, dst, np_dt):
    """Per-core idx stream (int16, 16-wrapped+replicated) and selector S."""
    out = []
    totslots = plan.totch * CHUNK
    for c in range(NCORES):
        ed, es, ps, half, sbid, m = groups[c]
        w = w_all[m]
        psl = np.where(half == 1, ps - HALFR, ps)
        key = sbid * 2 + half
        order = np.argsort(key, kind="stable")
        ed_s, psl_s, w_s, key_s = ed[order], psl[order], w[order], key[order]
        bounds = np.searchsorted(key_s, np.arange(2 * NSB + 1))
        idx_flat = np.zeros(totslots, np.int16)
        col_flat = np.zeros(totslots, np.int64)
        w_flat = np.zeros(totslots, np.float32)
        for sb in range(NSB):
            for h in (0, 1):
                g0, g1 = bounds[2 * sb + h], bounds[2 * sb + h + 1]
                n = g1 - g0
                if n == 0:
                    continue
                base = plan.slot_base[(sb, h)]
                idx_flat[base : base + n] = psl_s[g0:g1].astype(np.int16)
                col_flat[base : base + n] = ed_s[g0:g1] - sb * SEG
                w_flat[base : base + n] = w_s[g0:g1]
        wrapped = idx_flat.reshape(-1, 16).T.copy()      # [16, icols]
        idx_arr = np.tile(wrapped, (8, 1))               # replicate per Q7 core
        S = np.zeros((totslots, SEG), np.float32)
        S[np.arange(totslots), col_flat] = w_flat
        S = S.reshape(plan.totch, CHUNK, SEG).astype(np_dt)
        out.append((idx_arr, S))
    return out


def build_graph(plan, dt_name):
    import concourse.bacc as bacc
    import concourse.mybir as mybir
    import concourse.tile as tile

    f32 = mybir.dt.float32
    DT = mybir.dt.bfloat16 if dt_name == "bf16" else mybir.dt.float32
    chcap = max(c.nA + c.nB for c in plan.calls)

    nc = bacc.Bacc("TRN2", target_bir_lowering=False, debug=False,
                   num_devices=NCORES, num_swdge_queues=4)
    xs_p = nc.dram_tensor("xs", [RPAD, F], f32, kind="ExternalInput")
    idx_p = nc.dram_tensor("idx", [128, plan.icols], mybir.dt.int16,
                           kind="ExternalInput")
    sel_p = nc.dram_tensor("sel", [plan.totch, CHUNK, SEG], DT,
                           kind="ExternalInput")
    w1_p = nc.dram_tensor("w1", [F, H], f32, kind="ExternalInput")
    w2_p = nc.dram_tensor("w2", [H, F], f32, kind="ExternalInput")
    b1_p = nc.dram_tensor("b1", [2, 128], f32, kind="ExternalInput")
    b2_p = nc.dram_tensor("b2", [1, 128], f32, kind="ExternalInput")
    id_p = nc.dram_tensor("ident", [128, 128], f32, kind="ExternalInput")
    out_p = nc.dram_tensor("out", [RPAD, F], f32, kind="ExternalOutput")

    add = mybir.AluOpType.add
    mx = mybir.AluOpType.max
    rg = [list(range(NCORES))]

    with tile.TileContext(nc) as tc:
        with (
            tc.tile_pool(name="const", bufs=1) as constp,
            tc.tile_pool(name="res", bufs=1) as resp,
            tc.tile_pool(name="msg", bufs=4) as msgp,
            tc.tile_pool(name="selp", bufs=4) as selpool,
            tc.tile_pool(name="stage", bufs=2) as stagep,
            tc.tile_pool(name="ps_mp", bufs=2, space="PSUM") as psmp,
            tc.tile_pool(name="ps_w", bufs=3, space="PSUM") as pswp,
            tc.tile_pool(name="ps_t", bufs=1, space="PSUM") as pstp,
            tc.tile_pool(name="dram", bufs=1, space="DRAM") as dram,
        ):
            # ---- constants / resident tensors ----
            xs_t = resp.tile([128, 49, F], f32, tag="xs")
            nc.sync.dma_start(xs_t[:], xs_p.ap().rearrange("(c p) f -> p c f", p=128))
            idx_t = resp.tile([128, plan.icols], mybir.dt.int16, tag="idx")
            nc.sync.dma_start(idx_t[:], idx_p[:, :])
            w1f = constp.tile([F, H], f32, tag="w1f")
            nc.sync.dma_start(w1f[:], w1_p[:, :])
            w2f = constp.tile([128, 2, 128], f32, tag="w2f")
            nc.sync.dma_start(w2f[:], w2_p.ap().rearrange("(s k) m -> k s m", k=128))
            b1c = constp.tile([128, 2], f32, tag="b1")
            nc.sync.dma_start(b1c[:], b1_p.ap().rearrange("h p -> p h"))
            b2c = constp.tile([128, 1], f32, tag="b2")
            nc.sync.dma_start(b2c[:], b2_p.ap().rearrange("h p -> p h"))
            idf = constp.tile([128, 128], f32, tag="idf")
            nc.sync.dma_start(idf[:], id_p[:, :])
            if DT != f32:
                w1d = constp.tile([F, H], DT, tag="w1d")
                nc.vector.tensor_copy(w1d[:], w1f[:])
                w2d = constp.tile([128, 2, 128], DT, tag="w2d")
                nc.vector.tensor_copy(w2d[:], w2f[:])
                idd = constp.tile([128, 128], DT, tag="idd")
                nc.vector.tensor_copy(idd[:], idf[:])
            else:
                w1d, w2d, idd = w1f, w2f, idf

            bounce1 = dram.tile([RPAD, F], DT, tag="bounce1")
            table1 = dram.tile([NROWT, F], DT, tag="table1", addr_space="Shared")
            bounce2 = dram.tile([RPAD, F], DT, tag="bounce2")
            table2 = dram.tile([NROWT, F], DT, tag="table2", addr_space="Shared")

            # bounce1 = cast(xs)
            xbf = resp.tile([128, 49, F], DT, tag="xbf")
            nc.vector.tensor_copy(xbf[:], xs_t[:])
            nc.sync.dma_start(
                bounce1[:, :].rearrange("(c p) f -> p c f", p=128), xbf[:]
            )
            nc.gpsimd.collective_compute(
                "AllGather", mybir.AluOpType.bypass, replica_groups=rg,
                ins=[bounce1.opt()], outs=[table1.opt()],
            )

            limit = int(os.environ.get("GNN_LIMIT", "9999"))
            no_gather = os.environ.get("GNN_NO_GATHER", "0") == "1"
            no_mm = os.environ.get("GNN_NO_MM", "0") == "1"

            gctr = [0]

            def msgpass(table, layer):
                tabA = table[0:HALFR, :]
                tabB = table[HALFR:NROWT, :]
                for k, call in enumerate(plan.calls):
                    if k >= limit:
                        break
                    nch = call.nA + call.nB
                    ecols = call.evict_cols
                    nct = ecols // 128  # output 128-col chunks (2, last call 1)
                    msg = msgp.tile([128, chcap, F], DT, tag="msg")
                    st = selpool.tile([128, chcap, SEG], DT, tag="sel")
                    nc.sync.dma_start(
                        st[:, 0:nch, :],
                        sel_p[call.ch_off : call.ch_off + nch, :, :].rearrange(
                            "t p s -> p t s"
                        ),
                    )
                    if call.nA and not no_gather:
                        nc.gpsimd.dma_gather(
                            out_ap=msg[:, 0 : call.nA, :],
                            in_ap=tabA,
                            idxs_ap=idx_t[
                                :, call.idx_off // 16 : (call.idx_off + call.nA * 128) // 16
                            ],
                            num_idxs=call.nA * 128,
                            num_idxs_reg=call.nA * 128,
                            elem_size=F,
                            single_packet=call.nA * 128 <= 1024,
                            queue_num=gctr[0] % 4,
                        )
                        gctr[0] += 1
                    if call.nB and not no_gather:
                        boff = call.idx_off + call.nA * 128
                        nc.gpsimd.dma_gather(
                            out_ap=msg[:, call.nA : nch, :],
                            in_ap=tabB,
                            idxs_ap=idx_t[:, boff // 16 : (boff + call.nB * 128) // 16],
                            num_idxs=call.nB * 128,
                            num_idxs_reg=call.nB * 128,
                            elem_size=F,
                            single_packet=call.nB * 128 <= 1024,
                            queue_num=gctr[0] % 4,
                        )
                        gctr[0] += 1
                    ps = psmp.tile([128, CALL_COLS], f32, tag="mp")
                    if no_gather:
                        nc.vector.memset(msg[:, 0:nch, :], 0.25)
                    if no_mm:
                        nc.vector.memset(ps[:, :], 0.0)
                    for sb_i, slots in enumerate([] if no_mm else call.sb_slots):
                        for si, t in enumerate(slots):
                            nc.tensor.matmul(
                                ps[:, sb_i * SEG : (sb_i + 1) * SEG],
                                msg[:, t, :],
                                st[:, t, :],
                                start=(si == 0),
                                stop=(si == len(slots) - 1),
                            )
                    if layer == 1:
                        agg = stagep.tile([128, CALL_COLS], DT, tag="agg")
                        nc.any.tensor_copy(agg[:, :ecols], ps[:, :ecols])
                        h0 = stagep.tile([128, CALL_COLS], DT, tag="h0")
                        h1 = stagep.tile([128, CALL_COLS], DT, tag="h1")
                        for hf, ht in ((0, h0), (1, h1)):
                            wp = pswp.tile([128, CALL_COLS], f32, tag="wp")
                            nc.tensor.matmul(
                                wp[:, :ecols],
                                w1d[:, hf * 128 : (hf + 1) * 128],
                                agg[:, :ecols],
                                start=True,
                                stop=True,
                            )
                            nc.any.tensor_scalar(
                                ht[:, :ecols], wp[:, :ecols],
                                b1c[:, hf : hf + 1], 0.0, op0=add, op1=mx,
                            )
                        wp2 = pswp.tile([128, CALL_COLS], f32, tag="wp")
                        nc.tensor.matmul(
                            wp2[:, :ecols], w2d[:, 0, :], h0[:, :ecols],
                            start=True, stop=False,
                        )
                        nc.tensor.matmul(
                            wp2[:, :ecols], w2d[:, 1, :], h1[:, :ecols],
                            start=False, stop=True,
                        )
                        g = stagep.tile([128, CALL_COLS], DT, tag="g")
                        nc.any.tensor_copy(g[:, :ecols], wp2[:, :ecols])
                        gr = stagep.tile([128, SB_PER_CALL // 2, F], DT, tag="gr")
                        for ci in range(nct):
                            tp = pstp.tile([128, 128], DT, tag="tpd")
                            nc.tensor.transpose(
                                tp[:], g[:, ci * 128 : (ci + 1) * 128], idd[:]
                            )
                            nc.any.tensor_copy(gr[:, ci, :], tp[:])
                        nc.sync.dma_start(
                            bounce2[
                                k * CALL_COLS : k * CALL_COLS + ecols, :
                            ].rearrange("(c p) f -> p c f", p=128),
                            gr[:, 0:nct, :],
                        )
                    else:
                        a2 = stagep.tile([128, CALL_COLS], f32, tag="a2")
                        nc.any.tensor_scalar_add(
                            a2[:, :ecols], ps[:, :ecols], b2c[:, 0:1]
                        )
                        orow = stagep.tile([128, SB_PER_CALL // 2, F], f32, tag="or")
                        for ci in range(nct):
                            tp = pstp.tile([128, 128], f32, tag="tp")
                            nc.tensor.transpose(
                                tp[:], a2[:, ci * 128 : (ci + 1) * 128], idf[:]
                            )
                            cg = k * (CALL_COLS // 128) + ci
                            nc.vector.tensor_add(
                                orow[:, ci, :], tp[:], xs_t[:, cg, :]
                            )
                        nc.sync.dma_start(
                            out_p.ap()[
                                k * CALL_COLS : k * CALL_COLS + ecols, :
                            ].rearrange("(c p) f -> p c f", p=128),
                            orow[:, 0:nct, :],
                        )

            msgpass(table1, 1)
            nc.gpsimd.collective_compute(
                "AllGather", mybir.AluOpType.bypass, replica_groups=rg,
                ins=[bounce2.opt()], outs=[table2.opt()],
            )
            msgpass(table2, 2)

    nc.compile()
    return nc


def prepare(x, W1, b1, W2, b2, src, dst, dt_name=DT_NAME):
    import concourse.mybir as mybir

    np_dt = mybir.dt.np(
        mybir.dt.bfloat16 if dt_name == "bf16" else mybir.dt.float32
    )
    src = np.asarray(src).astype(np.int64)
    dst = np.asarray(dst).astype(np.int64)
    x = np.asarray(x, dtype=np.float32)
    norm_out, norm_in = _norms(src, dst)
    w_all = (norm_out[src] * norm_in[dst]).astype(np.float32)
    plan, groups = make_plan(src, dst)
    core_arrays = make_core_arrays(plan, groups, w_all, src, dst, np_dt)

    W1 = np.asarray(W1, dtype=np.float32)
    W2 = np.asarray(W2, dtype=np.float32)
    b1 = np.asarray(b1, dtype=np.float32).reshape(2, 128)
    b2 = np.asarray(b2, dtype=np.float32).reshape(1, 128)
    ident = np.eye(128, dtype=np.float32)

    in_maps = []
    for c in range(NCORES):
        xs = np.zeros((RPAD, F), np.float32)
        xs[:RPC] = x[c * RPC : (c + 1) * RPC]
        idx_arr, S = core_arrays[c]
        in_maps.append(
            {
                "xs": xs,
                "idx": idx_arr,
                "sel": S,
                "w1": W1,
                "w2": W2,
                "b1": b1,
                "b2": b2,
                "ident": ident,
            }
        )
    return plan, in_maps


_CACHE = {}


def run(x, W1, b1, W2, b2, src, dst, trace=False, dt_name=DT_NAME):
    from concourse import bass_utils

    key = (int(np.asarray(src)[0]), int(np.asarray(dst)[-1]), dt_name)
    plan, in_maps = prepare(x, W1, b1, W2, b2, src, dst, dt_name)
    if key not in _CACHE:
        _CACHE[key] = build_graph(plan, dt_name)
    nc = _CACHE[key]
    res = bass_utils.run_bass_kernel_spmd(
        nc, in_maps, core_ids=list(range(NCORES)), trace=trace
    )
    out = np.concatenate([res.results[c]["out"][:RPC] for c in range(NCORES)])
    return out.astype(np.float32), res.exec_time_ns


def kernel(x, W1, b1, W2, b2, src, dst):
    out, _ = run(x, W1, b1, W2, b2, src, dst, trace=False)
    return out


# revision 9
# speedup vs baseline: 1.6920x; 1.1757x over previous
"""Distributed Trainium2 kernel for the 2-layer GraphConv network.

Strategy (graph/data parallel, dst-partitioned):
- Host: compute degree norms from src/dst (integer index data), fold ALL
  normalization into per-edge weights w = norm_out[src]*norm_in[dst], which
  live in host-built per-chunk selector matrices S.  Edges are assigned to
  the core owning their dst node, grouped into 64-dst "sub-blocks", split by
  src table half (int16 DMA-gather index limit), and padded to 128-edge
  chunks with chunk budgets uniform across cores (SPMD: one instruction
  stream for all 8 cores).
- Device: AllGather the bf16 node-feature table; per call, dma_gather message
  rows, multiply msg_chunk^T @ S on the TensorEngine accumulating agg^T tiles
  in PSUM (this performs the segment-sum aggregation), then apply the dense
  layers (W1+bias+relu, W2) per 256-column chunk, transpose, AllGather the
  layer-2 table, repeat message passing, add b2 + residual x, write out.
"""

import os
import sys

import numpy as np

sys.path.insert(0, os.path.dirname(os.path.abspath(__file__)))

N = 50000
E = 800000
F = 128
H = 256
NCORES = 8
RPC = N // NCORES          # 6250 rows per core
CHUNK = 128
RPAD = 6272                # 49 * 128, per-core padded row count
NROWT = RPAD * NCORES      # 50176 rows in the gathered table
HALFR = NROWT // 2         # 25088, int16-addressable half
SEG = 64                   # dst slots per sub-block
SB_PER_CALL = 4
CALL_COLS = SEG * SB_PER_CALL   # 256
NSB = RPAD // SEG          # 98 sub-blocks per core
NCALLS = (NSB + SB_PER_CALL - 1) // SB_PER_CALL  # 25

DT_NAME = os.environ.get("GNN_DT", "bf16")


class Call:
    __slots__ = ("nA", "nB", "idx_off", "ch_off", "sb_slots", "evict_cols")


class Plan:
    __slots__ = ("calls", "totch", "icols", "slot_base", "budA", "budB")


def _norms(src, dst):
    deg_out = np.bincount(src, minlength=N).astype(np.float32)
    deg_in = np.bincount(dst, minlength=N).astype(np.float32)
    norm_out = (1.0 / np.sqrt(np.clip(deg_out, 1.0, None))).astype(np.float32)
    norm_in = (1.0 / np.sqrt(np.clip(deg_in, 1.0, None))).astype(np.float32)
    return norm_out, norm_in


def make_plan(src, dst):
    """Compute uniform (cross-core) chunk budgets and the call schedule."""
    src = np.asarray(src).astype(np.int64)
    dst = np.asarray(dst).astype(np.int64)
    owner = dst // RPC
    budA = np.zeros(NSB, np.int64)
    budB = np.zeros(NSB, np.int64)
    per_core_groups = []
    for c in range(NCORES):
        m = owner == c
        ed = dst[m] - c * RPC
        es = src[m]
        ps = (es // RPC) * RPAD + (es % RPC)
        half = (ps >= HALFR).astype(np.int64)
        sbid = ed // SEG
        cntA = np.bincount(sbid[half == 0], minlength=NSB)
        cntB = np.bincount(sbid[half == 1], minlength=NSB)
        budA = np.maximum(budA, -(-cntA // CHUNK))
        budB = np.maximum(budB, -(-cntB // CHUNK))
        per_core_groups.append((ed, es, ps, half, sbid, m))
    # make sure every sub-block has at least one chunk so its PSUM region is
    # written (start=True zeroes it) even when no core has edges there
    empty = (budA + budB) == 0
    budA[empty] = 1

    plan = Plan()
    plan.budA, plan.budB = budA, budB
    plan.calls = []
    plan.slot_base = {}
    slot = 0
    choff = 0
    for k in range(NCALLS):
        sbs = list(range(k * SB_PER_CALL, min((k + 1) * SB_PER_CALL, NSB)))
        call = Call()
        call.idx_off = slot
        call.ch_off = choff
        call.evict_cols = len(sbs) * SEG
        a_slots = {}
        local = 0
        for sb in sbs:
            plan.slot_base[(sb, 0)] = slot + local * CHUNK
            a_slots[sb] = list(range(local, local + budA[sb]))
            local += budA[sb]
        call.nA = local
        for sb in sbs:
            plan.slot_base[(sb, 1)] = slot + local * CHUNK
            a_slots[sb] = a_slots[sb] + list(range(local, local + budB[sb]))
            local += budB[sb]
        call.nB = local - call.nA
        call.sb_slots = [a_slots[sb] for sb in sbs]
        slot += local * CHUNK
        choff += local
        plan.calls.append(call)
    plan.totch = choff
    plan.icols = slot // 16
    return plan, per_core_groups


def make_core_arrays(plan, groups, w_all, src, dst, np_dt):
    """Per-core idx stream (int16, 16-wrapped+replicated) and selector S."""
    out = []
    totslots = plan.totch * CHUNK
    for c in range(NCORES):
        ed, es, ps, half, sbid, m = groups[c]
        w = w_all[m]
        psl = np.where(half == 1, ps - HALFR, ps)
        key = sbid * 2 + half
        order = np.argsort(key, kind="stable")
        ed_s, psl_s, w_s, key_s = ed[order], psl[order], w[order], key[order]
        bounds = np.searchsorted(key_s, np.arange(2 * NSB + 1))
        idx_flat = np.zeros(totslots, np.int16)
        col_flat = np.zeros(totslots, np.int64)
        w_flat = np.zeros(totslots, np.float32)
        for sb in range(NSB):
            for h in (0, 1):
                g0, g1 = bounds[2 * sb + h], bounds[2 * sb + h + 1]
                n = g1 - g0
                if n == 0:
                    continue
                base = plan.slot_base[(sb, h)]
                idx_flat[base : base + n] = psl_s[g0:g1].astype(np.int16)
                col_flat[base : base + n] = ed_s[g0:g1] - sb * SEG
                w_flat[base : base + n] = w_s[g0:g1]
        wrapped = idx_flat.reshape(-1, 16).T.copy()      # [16, icols]
        idx_arr = np.tile(wrapped, (8, 1))               # replicate per Q7 core
        S = np.zeros((totslots, SEG), np.float32)
        S[np.arange(totslots), col_flat] = w_flat
        # [totch, 128, SEG] -> [128, totch*SEG] partition-major for fast DMA
        S = (
            S.reshape(plan.totch, CHUNK, SEG)
            .transpose(1, 0, 2)
            .reshape(CHUNK, plan.totch * SEG)
            .astype(np_dt)
        )
        out.append((idx_arr, S))
    return out


def build_graph(plan, dt_name):
    import concourse.bacc as bacc
    import concourse.mybir as mybir
    import concourse.tile as tile

    f32 = mybir.dt.float32
    DT = mybir.dt.bfloat16 if dt_name == "bf16" else mybir.dt.float32
    chcap = max(c.nA + c.nB for c in plan.calls)

    nc = bacc.Bacc("TRN2", target_bir_lowering=False, debug=False,
                   num_devices=NCORES, num_swdge_queues=4)
    xs_p = nc.dram_tensor("xs", [RPAD, F], f32, kind="ExternalInput")
    idx_p = nc.dram_tensor("idx", [128, plan.icols], mybir.dt.int16,
                           kind="ExternalInput")
    sel_p = nc.dram_tensor("sel", [CHUNK, plan.totch * SEG], DT,
                           kind="ExternalInput")
    w1_p = nc.dram_tensor("w1", [F, H], f32, kind="ExternalInput")
    w2_p = nc.dram_tensor("w2", [H, F], f32, kind="ExternalInput")
    b1_p = nc.dram_tensor("b1", [2, 128], f32, kind="ExternalInput")
    b2_p = nc.dram_tensor("b2", [1, 128], f32, kind="ExternalInput")
    id_p = nc.dram_tensor("ident", [128, 128], f32, kind="ExternalInput")
    out_p = nc.dram_tensor("out", [RPAD, F], f32, kind="ExternalOutput")

    add = mybir.AluOpType.add
    mx = mybir.AluOpType.max
    rg = [list(range(NCORES))]

    with tile.TileContext(nc) as tc:
        with (
            tc.tile_pool(name="const", bufs=1) as constp,
            tc.tile_pool(name="res", bufs=1) as resp,
            tc.tile_pool(name="msg", bufs=4) as msgp,
            tc.tile_pool(name="selp", bufs=4) as selpool,
            tc.tile_pool(name="stage", bufs=2) as stagep,
            tc.tile_pool(name="ps_mp", bufs=2, space="PSUM") as psmp,
            tc.tile_pool(name="ps_w", bufs=3, space="PSUM") as pswp,
            tc.tile_pool(name="ps_t", bufs=1, space="PSUM") as pstp,
            tc.tile_pool(name="dram", bufs=1, space="DRAM") as dram,
        ):
            # ---- constants / resident tensors ----
            xs_t = resp.tile([128, 49, F], f32, tag="xs")
            nc.sync.dma_start(xs_t[:], xs_p.ap().rearrange("(c p) f -> p c f", p=128))
            idx_t = resp.tile([128, plan.icols], mybir.dt.int16, tag="idx")
            nc.sync.dma_start(idx_t[:], idx_p[:, :])
            w1f = constp.tile([F, H], f32, tag="w1f")
            nc.sync.dma_start(w1f[:], w1_p[:, :])
            w2f = constp.tile([128, 2, 128], f32, tag="w2f")
            nc.sync.dma_start(w2f[:], w2_p.ap().rearrange("(s k) m -> k s m", k=128))
            b1c = constp.tile([128, 2], f32, tag="b1")
            nc.sync.dma_start(b1c[:], b1_p.ap().rearrange("h p -> p h"))
            b2c = constp.tile([128, 1], f32, tag="b2")
            nc.sync.dma_start(b2c[:], b2_p.ap().rearrange("h p -> p h"))
            idf = constp.tile([128, 128], f32, tag="idf")
            nc.sync.dma_start(idf[:], id_p[:, :])
            if DT != f32:
                w1d = constp.tile([F, H], DT, tag="w1d")
                nc.vector.tensor_copy(w1d[:], w1f[:])
                w2d = constp.tile([128, 2, 128], DT, tag="w2d")
                nc.vector.tensor_copy(w2d[:], w2f[:])
                idd = constp.tile([128, 128], DT, tag="idd")
                nc.vector.tensor_copy(idd[:], idf[:])
            else:
                w1d, w2d, idd = w1f, w2f, idf

            bounce1 = dram.tile([RPAD, F], DT, tag="bounce1")
            table1 = dram.tile([NROWT, F], DT, tag="table1", addr_space="Shared")
            bounce2 = dram.tile([RPAD, F], DT, tag="bounce2")
            table2 = dram.tile([NROWT, F], DT, tag="table2", addr_space="Shared")

            # bounce1 = cast(xs)
            xbf = resp.tile([128, 49, F], DT, tag="xbf")
            nc.vector.tensor_copy(xbf[:], xs_t[:])
            nc.sync.dma_start(
                bounce1[:, :].rearrange("(c p) f -> p c f", p=128), xbf[:]
            )
            nc.gpsimd.collective_compute(
                "AllGather", mybir.AluOpType.bypass, replica_groups=rg,
                ins=[bounce1.opt()], outs=[table1.opt()],
            )

            limit = int(os.environ.get("GNN_LIMIT", "9999"))
            no_gather = os.environ.get("GNN_NO_GATHER", "0") == "1"
            no_mm = os.environ.get("GNN_NO_MM", "0") == "1"

            gctr = [0]

            def msgpass(table, layer):
                tabA = table[0:HALFR, :]
                tabB = table[HALFR:NROWT, :]
                for k, call in enumerate(plan.calls):
                    if k >= limit:
                        break
                    nch = call.nA + call.nB
                    ecols = call.evict_cols
                    nct = ecols // 128  # output 128-col chunks (2, last call 1)
                    msg = msgp.tile([128, chcap, F], DT, tag="msg")
                    st = selpool.tile([128, chcap, SEG], DT, tag="sel")
                    nc.sync.dma_start(
                        st[:, 0:nch, :],
                        sel_p[:, call.ch_off * SEG : (call.ch_off + nch) * SEG],
                    )
                    if call.nA and not no_gather:
                        nc.gpsimd.dma_gather(
                            out_ap=msg[:, 0 : call.nA, :],
                            in_ap=tabA,
                            idxs_ap=idx_t[
                                :, call.idx_off // 16 : (call.idx_off + call.nA * 128) // 16
                            ],
                            num_idxs=call.nA * 128,
                            num_idxs_reg=call.nA * 128,
                            elem_size=F,
                            single_packet=call.nA * 128 <= 1024,
                            queue_num=gctr[0] % 4,
                        )
                        gctr[0] += 1
                    if call.nB and not no_gather:
                        boff = call.idx_off + call.nA * 128
                        nc.gpsimd.dma_gather(
                            out_ap=msg[:, call.nA : nch, :],
                            in_ap=tabB,
                            idxs_ap=idx_t[:, boff // 16 : (boff + call.nB * 128) // 16],
                            num_idxs=call.nB * 128,
                            num_idxs_reg=call.nB * 128,
                            elem_size=F,
                            single_packet=call.nB * 128 <= 1024,
                            queue_num=gctr[0] % 4,
                        )
                        gctr[0] += 1
                    ps = psmp.tile([128, CALL_COLS], f32, tag="mp")
                    if no_gather:
                        nc.vector.memset(msg[:, 0:nch, :], 0.25)
                    if no_mm:
                        nc.vector.memset(ps[:, :], 0.0)
                    for sb_i, slots in enumerate([] if no_mm else call.sb_slots):
                        for si, t in enumerate(slots):
                            nc.tensor.matmul(
                                ps[:, sb_i * SEG : (sb_i + 1) * SEG],
                                msg[:, t, :],
                                st[:, t, :],
                                start=(si == 0),
                                stop=(si == len(slots) - 1),
                            )
                    if layer == 1:
                        agg = stagep.tile([128, CALL_COLS], DT, tag="agg")
                        nc.any.tensor_copy(agg[:, :ecols], ps[:, :ecols])
                        h0 = stagep.tile([128, CALL_COLS], DT, tag="h0")
                        h1 = stagep.tile([128, CALL_COLS], DT, tag="h1")
                        for hf, ht in ((0, h0), (1, h1)):
                            wp = pswp.tile([128, CALL_COLS], f32, tag="wp")
                            nc.tensor.matmul(
                                wp[:, :ecols],
                                w1d[:, hf * 128 : (hf + 1) * 128],
                                agg[:, :ecols],
                                start=True,
                                stop=True,
                            )
                            nc.any.tensor_scalar(
                                ht[:, :ecols], wp[:, :ecols],
                                b1c[:, hf : hf + 1], 0.0, op0=add, op1=mx,
                            )
                        wp2 = pswp.tile([128, CALL_COLS], f32, tag="wp")
                        nc.tensor.matmul(
                            wp2[:, :ecols], w2d[:, 0, :], h0[:, :ecols],
                            start=True, stop=False,
                        )
                        nc.tensor.matmul(
                            wp2[:, :ecols], w2d[:, 1, :], h1[:, :ecols],
                            start=False, stop=True,
                        )
                        g = stagep.tile([128, CALL_COLS], DT, tag="g")
                        nc.any.tensor_copy(g[:, :ecols], wp2[:, :ecols])
                        gr = stagep.tile([128, SB_PER_CALL // 2, F], DT, tag="gr")
                        for ci in range(nct):
                            tp = pstp.tile([128, 128], DT, tag="tpd")
                            nc.tensor.transpose(
                                tp[:], g[:, ci * 128 : (ci + 1) * 128], idd[:]
                            )
                            nc.any.tensor_copy(gr[:, ci, :], tp[:])
                        nc.sync.dma_start(
                            bounce2[
                                k * CALL_COLS : k * CALL_COLS + ecols, :
                            ].rearrange("(c p) f -> p c f", p=128),
                            gr[:, 0:nct, :],
                        )
                    else:
                        a2 = stagep.tile([128, CALL_COLS], f32, tag="a2")
                        nc.any.tensor_scalar_add(
                            a2[:, :ecols], ps[:, :ecols], b2c[:, 0:1]
                        )
                        orow = stagep.tile([128, SB_PER_CALL // 2, F], f32, tag="or")
                        for ci in range(nct):
                            tp = pstp.tile([128, 128], f32, tag="tp")
                            nc.tensor.transpose(
                                tp[:], a2[:, ci * 128 : (ci + 1) * 128], idf[:]
                            )
                            cg = k * (CALL_COLS // 128) + ci
                            nc.vector.tensor_add(
                                orow[:, ci, :], tp[:], xs_t[:, cg, :]
                            )
                        nc.sync.dma_start(
                            out_p.ap()[
                                k * CALL_COLS : k * CALL_COLS + ecols, :
                            ].rearrange("(c p) f -> p c f", p=128),
                            orow[:, 0:nct, :],
                        )

            msgpass(table1, 1)
            nc.gpsimd.collective_compute(
                "AllGather", mybir.AluOpType.bypass, replica_groups=rg,
                ins=[bounce2.opt()], outs=[table2.opt()],
            )
            msgpass(table2, 2)

    nc.compile()
    return nc


def prepare(x, W1, b1, W2, b2, src, dst, dt_name=DT_NAME):
    import concourse.mybir as mybir

    np_dt = mybir.dt.np(
        mybir.dt.bfloat16 if dt_name == "bf16" else mybir.dt.float32
    )
    src = np.asarray(src).astype(np.int64)
    dst = np.asarray(dst).astype(np.int64)
    x = np.asarray(x, dtype=np.float32)
    norm_out, norm_in = _norms(src, dst)
    w_all = (norm_out[src] * norm_in[dst]).astype(np.float32)
    plan, groups = make_plan(src, dst)
    core_arrays = make_core_arrays(plan, groups, w_all, src, dst, np_dt)

    W1 = np.asarray(W1, dtype=np.float32)
    W2 = np.asarray(W2, dtype=np.float32)
    b1 = np.asarray(b1, dtype=np.float32).reshape(2, 128)
    b2 = np.asarray(b2, dtype=np.float32).reshape(1, 128)
    ident = np.eye(128, dtype=np.float32)

    in_maps = []
    for c in range(NCORES):
        xs = np.zeros((RPAD, F), np.float32)
        xs[:RPC] = x[c * RPC : (c + 1) * RPC]
        idx_arr, S = core_arrays[c]
        in_maps.append(
            {
                "xs": xs,
                "idx": idx_arr,
                "sel": S,
                "w1": W1,
                "w2": W2,
                "b1": b1,
                "b2": b2,
                "ident": ident,
            }
        )
    return plan, in_maps


_CACHE = {}


def run(x, W1, b1, W2, b2, src, dst, trace=False, dt_name=DT_NAME):
    from concourse import bass_utils

    key = (int(np.asarray(src)[0]), int(np.asarray(dst)[-1]), dt_name)
    plan, in_maps = prepare(x, W1, b1, W2, b2, src, dst, dt_name)
    if key not in _CACHE:
        _CACHE[key] = build_graph(plan, dt_name)
    nc = _CACHE[key]
    res = bass_utils.run_bass_kernel_spmd(
        nc, in_maps, core_ids=list(range(NCORES)), trace=trace
    )
    out = np.concatenate([res.results[c]["out"][:RPC] for c in range(NCORES)])
    return out.astype(np.float32), res.exec_time_ns


def kernel(x, W1, b1, W2, b2, src, dst):
    out, _ = run(x, W1, b1, W2, b2, src, dst, trace=False)
    return out
